# revision 42
# baseline (speedup 1.0000x reference)
"""Trainium2 Bass kernel for nn_BILINEAR_56169582297414 (gnn message passing).

Reference computation (per prediction pair b):
    item_e = item_table[item_inputs[b]]                    # [D]
    mem_e  = user_table[member_ids[b, :]]                  # [M, D]
    scores[m] = mem_e[m] @ W_bil @ item_e + b_bil          # bilinear
    w = scores * member_mask[b]                            # mask padded members
    fu = sum_m w[m] * mem_e[m]                             # [D]
    ne = [fu * item_e, fu, item_e]                         # [3D]
    y = sigmoid(relu(ne @ W1.T + b1) @ W2.T + b2)          # [1]

Strategy: data-parallel over 8 NeuronCores, tables replicated. The serial
resource is SWDGE descriptor generation on the GPSIMD (Pool) engine; each
dma_gather runs on ONE Q7 core-pair selected by queue_num, so gathers for 4
consecutive tiles are issued back-to-back on queues 0-3 to run on all 4
core-pairs concurrently.  Rows are sorted by true group length (desc) and
striped across cores so each tile fetches only maxL members.  Tables are
bf16 4-packed (256B gather elements, idx = id>>2 fits int16); the 1-of-4
sub-row select uses DVE copy_predicated with host-precomputed masks.
Scores/weighted-sum run on DVE in bf16 with a contiguous halving tree for
the member reduction; bilinear projection + MLP head run on TensorE with
batched (per-4-tile) matmuls.
"""

import sys

sys.path.insert(0, "/opt/trn_rl_repo")

import numpy as np

B = 262144
M = 16
NU = 100000
NI = 50000
D = 32
N_CORES = 8
BC = B // N_CORES
P = 128
NT = BC // P
G = 4  # tiles per group: one member-gather instruction per group, 4 queues

_COMPILED = {}


def _group_gl(prof, g=G):
    """Per-group max member count (prof is non-increasing)."""
    return [max(prof[i * g : (i + 1) * g]) for i in range(len(prof) // g)]


def _tree_steps(gl):
    """Halving-tree add steps for ragged length gl -> [(dst_len, src_off)]."""
    steps = []
    n = gl
    while n > 1:
        h = (n + 1) // 2
        steps.append((n - h, h))  # add cols [h, n) onto [0, n-h)
        n = h
    return steps


def build_kernel(bc, g=G, prof=None):
    """Per-core Bass program against bf16 4-packed tables user4 [25000, 128]
    and item4 [12500, 128]."""
    import concourse.bacc as bacc
    import concourse.tile as tile
    from concourse import mybir
    from concourse.library_config import mlp

    nt = bc // P
    assert nt % g == 0
    ngroups = nt // g
    if prof is None:
        prof = [M] * nt
    prof = [int(max(1, min(M, x))) for x in prof]
    gls = _group_gl(prof, g)
    dt = mybir.dt
    bf = dt.bfloat16

    # flat col offsets for per-group idx/mask tensors
    idx_cols = [g * gl * 8 for gl in gls]       # int16 cols ([128, .])
    jm_cols = [g * gl for gl in gls]            # member-mask cols
    idx_off = np.concatenate([[0], np.cumsum(idx_cols)]).astype(int)
    jm_off = np.concatenate([[0], np.cumsum(jm_cols)]).astype(int)

    nc = bacc.Bacc(
        "TRN2", target_bir_lowering=False, debug=False, num_swdge_queues=4
    )

    ids16 = nc.dram_tensor("ids16", [P, int(idx_off[-1])], dt.int16,
                           kind="ExternalInput")
    iid16 = nc.dram_tensor("iid16", [P, ngroups * g * 8], dt.int16,
                           kind="ExternalInput")
    msel = [
        nc.dram_tensor(f"msel{q}", [P, int(jm_off[-1])], dt.uint8,
                       kind="ExternalInput")
        for q in ("23", "odd")
    ]
    isel = [
        nc.dram_tensor(f"isel{q}", [P, ngroups * g], dt.uint8,
                       kind="ExternalInput")
        for q in ("23", "odd")
    ]
    mask = nc.dram_tensor("mask", [bc, M], bf, kind="ExternalInput")
    user4 = nc.dram_tensor("user4", [NU // 4, 4 * D], bf, kind="ExternalInput")
    item4 = nc.dram_tensor("item4", [NI // 4, 4 * D], bf, kind="ExternalInput")
    w_bil_t = nc.dram_tensor("w_bil_t", [D, D], bf, kind="ExternalInput")
    w1_t = nc.dram_tensor("w1_t", [3 * D, 8], bf, kind="ExternalInput")
    w2_t = nc.dram_tensor("w2_t", [8, 1], bf, kind="ExternalInput")
    b1 = nc.dram_tensor("b1", [8, 1], dt.float32, kind="ExternalInput")
    b2 = nc.dram_tensor("b2", [1, 1], dt.float32, kind="ExternalInput")
    bbil = nc.dram_tensor("bbil", [P, 1], dt.float32, kind="ExternalInput")
    ident = nc.dram_tensor("ident", [P, P], bf, kind="ExternalInput")
    y_out = nc.dram_tensor("y", [nt, P], dt.float32, kind="ExternalOutput")

    GM = g * M
    GP = g * P
    SB = 4  # groups per issue batch (one per SWDGE queue)
    assert ngroups % SB == 0
    nbatch = ngroups // SB

    with tile.TileContext(nc) as tc:
        with (
            tc.tile_pool(name="const", bufs=1) as cpool,
            tc.tile_pool(name="io", bufs=5) as iopool,
            tc.tile_pool(name="work", bufs=3) as wpool,
            tc.tile_pool(name="gath", bufs=2) as gpool,
            tc.tile_pool(name="gath2", bufs=2) as g2pool,
            tc.tile_pool(name="prodp", bufs=2) as prpool,
            tc.tile_pool(name="psum", bufs=1, space="PSUM") as ppool,
            tc.tile_pool(name="psumv", bufs=2, space="PSUM") as ppoolv,
        ):
            with tc.tile_critical():
                nc.gpsimd.load_library(mlp)

            wt_sb = cpool.tile([D, D], bf, tag="wt")
            nc.sync.dma_start(out=wt_sb[:], in_=w_bil_t[:])
            w1_sb = cpool.tile([3 * D, 8], bf, tag="w1")
            nc.sync.dma_start(out=w1_sb[:], in_=w1_t[:])
            w2_sb = cpool.tile([8, 1], bf, tag="w2")
            nc.sync.dma_start(out=w2_sb[:], in_=w2_t[:])
            b1_sb = cpool.tile([8, 1], dt.float32, tag="b1")
            nc.sync.dma_start(out=b1_sb[:], in_=b1[:])
            b2_sb = cpool.tile([1, 1], dt.float32, tag="b2")
            nc.sync.dma_start(out=b2_sb[:], in_=b2[:])
            bbil_sb = cpool.tile([P, 1], dt.float32, tag="bbil")
            nc.sync.dma_start(out=bbil_sb[:], in_=bbil[:])
            id_sb = cpool.tile([P, P], bf, tag="ident")
            nc.sync.dma_start(out=id_sb[:], in_=ident[:])

            def load_batch(bj):
                """Issue batched input DMAs for batch bj; returns tile dict."""
                h0 = bj * SB
                i0, i1 = int(idx_off[h0]), int(idx_off[h0 + SB])
                j0, j1 = int(jm_off[h0]), int(jm_off[h0 + SB])
                ids_b = iopool.tile([P, i1 - i0], dt.int16, tag="ids")
                nc.sync.dma_start(out=ids_b[:], in_=ids16[:, i0:i1])
                ms_b = []
                for s in range(2):
                    m_sb = iopool.tile([P, j1 - j0], dt.uint8, tag=f"ms{s}")
                    nc.sync.dma_start(out=m_sb[:], in_=msel[s][:, j0:j1])
                    ms_b.append(m_sb)
                mask_b = iopool.tile([P, SB * GM], bf, tag="mask")
                nc.sync.dma_start(
                    out=mask_b[:].rearrange("p (q g m) -> p q g m", q=SB, g=g),
                    in_=mask[h0 * GP : (h0 + SB) * GP, :]
                        .rearrange("(q g p) m -> p q g m", p=P, g=g),
                )
                iid_b = iopool.tile([P, SB * g * 8], dt.int16, tag="iid")
                nc.sync.dma_start(
                    out=iid_b[:],
                    in_=iid16[:, h0 * g * 8 : (h0 + SB) * g * 8],
                )
                is_b = []
                for s in range(2):
                    i_sb = iopool.tile([P, SB * g], dt.uint8, tag=f"is{s}")
                    nc.sync.dma_start(
                        out=i_sb[:], in_=isel[s][:, h0 * g : (h0 + SB) * g]
                    )
                    is_b.append(i_sb)
                return {"ids": ids_b, "ms": ms_b, "mask": mask_b,
                        "iid": iid_b, "is": is_b, "i0": i0, "j0": j0}

            LA = 2  # input-load lookahead (batches)
            loaded = {}
            for bj in range(min(LA + 1, nbatch)):
                loaded[bj] = load_batch(bj)

            for bi in range(nbatch):
                g0 = bi * SB
                bgls = gls[g0 : g0 + SB]
                if bi + LA + 1 < nbatch:
                    loaded[bi + LA + 1] = load_batch(bi + LA + 1)
                bt = loaded.pop(bi)
                iid_sb = bt["iid"]
                is_sbs = bt["is"]

                # --- one batched item gather first (small, 16 tiles) -----
                g2_sb = g2pool.tile([P, SB * g * 4 * D], bf, tag="g2")
                g2 = g2_sb[:].rearrange("p (c e) -> p c e", c=SB * g)
                nc.gpsimd.dma_gather(
                    out_ap=g2,
                    in_ap=item4[:],
                    idxs_ap=iid_sb[:],
                    num_idxs=SB * g * 128,
                    num_idxs_reg=SB * g * 128,
                    elem_size=4 * D,
                    single_packet=False,
                    queue_num=bi % 4,
                )

                # --- 4 member gathers, queues 0-3, back to back ----------
                g4s = []
                for q in range(SB):
                    gi = g0 + q
                    GL = bgls[q]
                    nmi = g * GL * 128
                    ids_ap = bt["ids"][
                        :, int(idx_off[gi]) - bt["i0"]
                           : int(idx_off[gi + 1]) - bt["i0"]
                    ]
                    g4_sb = gpool.tile([P, g * GL * 4 * D], bf, tag=f"g4_{q}")
                    g4 = g4_sb[:].rearrange("p (c e) -> p c e", c=g * GL)
                    nc.gpsimd.dma_gather(
                        out_ap=g4,
                        in_ap=user4[:],
                        idxs_ap=ids_ap,
                        num_idxs=nmi,
                        num_idxs_reg=nmi,
                        elem_size=4 * D,
                        single_packet=False,
                        queue_num=q,
                    )
                    g4s.append(g4)

                # --- math per group --------------------------------------
                for q in range(SB):
                    gi = g0 + q
                    GL = bgls[q]
                    C = g * GL

                    # 1-of-4 member sub-row select, in place in g4 via a
                    # binary tree: pick the 2D pair by (sub>=2), then the
                    # D row by (sub&1).  mem stays at g4[:, :, 0:D].
                    g4 = g4s[q]
                    jlo = int(jm_off[gi]) - bt["j0"]
                    jhi = int(jm_off[gi + 1]) - bt["j0"]
                    m23_ap = bt["ms"][0][:, jlo:jhi]
                    modd_ap = bt["ms"][1][:, jlo:jhi]
                    nc.vector.copy_predicated(
                        out=g4[:, :, 0 : 2 * D],
                        mask=m23_ap.unsqueeze(2).broadcast_to([P, C, 2 * D]),
                        data=g4[:, :, 2 * D : 4 * D],
                    )
                    nc.vector.copy_predicated(
                        out=g4[:, :, 0:D],
                        mask=modd_ap.unsqueeze(2).broadcast_to([P, C, D]),
                        data=g4[:, :, D : 2 * D],
                    )
                    # compact selected rows out of g4 on the Scalar engine:
                    # frees the raw gather buffer early and gives the DVE
                    # multiplies contiguous reads
                    mem_sb = wpool.tile([P, C * D], bf, tag="mem")
                    nc.scalar.activation(
                        out=mem_sb[:].rearrange("p (c d) -> p c d", c=C),
                        in_=g4[:, :, 0:D],
                        func=mybir.ActivationFunctionType.Copy,
                    )

                    # item 1-of-4 select, in place in g2; scalar engine
                    # copies the result into ne[..., 2D:3D]
                    ne_sb = wpool.tile([P, g * 3 * D], bf, tag="ne")
                    ne3 = ne_sb[:].rearrange("p (g c) -> p g c", g=g)
                    g2q = g2[:, q * g : (q + 1) * g, :]
                    nc.vector.copy_predicated(
                        out=g2q[:, :, 0 : 2 * D],
                        mask=is_sbs[0][:, q * g : (q + 1) * g]
                             .unsqueeze(2).broadcast_to([P, g, 2 * D]),
                        data=g2q[:, :, 2 * D : 4 * D],
                    )
                    nc.vector.copy_predicated(
                        out=g2q[:, :, 0:D],
                        mask=is_sbs[1][:, q * g : (q + 1) * g]
                             .unsqueeze(2).broadcast_to([P, g, D]),
                        data=g2q[:, :, D : 2 * D],
                    )
                    nc.scalar.activation(
                        out=ne3[:, :, 2 * D : 3 * D],
                        in_=g2q[:, :, 0:D],
                        func=mybir.ActivationFunctionType.Copy,
                    )

                    # itemT via PE transpose (bf16 PSUM), then v = W^T @ item
                    itemT_ps = ppool.tile([D, GP], bf, tag="itemT",
                                          space="PSUM")
                    for j in range(g):
                        nc.tensor.transpose(
                            out=itemT_ps[:, j * P : (j + 1) * P],
                            in_=g2q[:, j, 0:D],
                            identity=id_sb[:],
                        )
                    itemT_sb = wpool.tile([D, GP], bf, tag="itemT")
                    nc.scalar.activation(
                        out=itemT_sb[:],
                        in_=itemT_ps[:],
                        func=mybir.ActivationFunctionType.Copy,
                    )

                    v_ps = ppoolv.tile([P, g * D], dt.float32, tag="v",
                                       space="PSUM")
                    for j in range(g):
                        nc.tensor.matmul(
                            v_ps[:, j * D : (j + 1) * D],
                            lhsT=itemT_sb[:, j * P : (j + 1) * P],
                            rhs=wt_sb[:],
                            start=True,
                            stop=True,
                        )
                    v_sb = wpool.tile([P, g * D], bf, tag="vsb")
                    nc.scalar.activation(
                        out=v_sb[:],
                        in_=v_ps[:],
                        func=mybir.ActivationFunctionType.Copy,
                    )

                    # scores = sum_d mem * v  (X-reduce over d)
                    mem4 = mem_sb[:].rearrange("p (g m d) -> p g m d",
                                               g=g, m=GL)
                    v_b = (
                        v_sb[:]
                        .rearrange("p (g d) -> p g d", g=g)
                        .unsqueeze(2)
                        .broadcast_to([P, g, GL, D])
                    )
                    prod_sb = prpool.tile([P, GM * D], bf, tag="prod")
                    prod4 = prod_sb[:].rearrange("p (g m d) -> p g m d",
                                                 g=g, m=M)[:, :, :GL, :]
                    nc.vector.tensor_mul(out=prod4, in0=mem4, in1=v_b)

                    scores_sb = wpool.tile([P, GM], dt.float32, tag="scores")
                    sc3 = scores_sb[:].rearrange("p (g m) -> p g m", g=g)
                    nc.vector.reduce_sum(
                        out=sc3[:, :, :GL], in_=prod4,
                        axis=mybir.AxisListType.X,
                    )

                    # w = (scores + b_bil) * mask
                    w_sb = wpool.tile([P, GM], bf, tag="w")
                    w3 = w_sb[:].rearrange("p (g m) -> p g m", g=g)
                    m3 = bt["mask"][:].rearrange(
                        "p (q g m) -> p q g m", q=SB, g=g
                    )[:, q]
                    nc.vector.scalar_tensor_tensor(
                        out=w3[:, :, :GL],
                        in0=sc3[:, :, :GL],
                        scalar=bbil_sb[:, :1],
                        in1=m3[:, :, :GL],
                        op0=mybir.AluOpType.add,
                        op1=mybir.AluOpType.mult,
                    )

                    # prod2 = w * mem, then halving-tree sum over m -> fu
                    w_b = w3[:, :, :GL].unsqueeze(3).broadcast_to([P, g, GL, D])
                    nc.vector.tensor_mul(out=prod4, in0=mem4, in1=w_b)

                    steps = _tree_steps(GL)
                    prod4 = prod_sb[:].rearrange("p (g m d) -> p g m d",
                                                 g=g, m=M)[:, :, :GL, :]
                    for si, (dlen, off) in enumerate(steps):
                        last = si == len(steps) - 1
                        dst = (ne3[:, :, D : 2 * D].unsqueeze(2)
                               if last and dlen == 1
                               else prod4[:, :, :dlen, :])
                        nc.vector.tensor_add(
                            out=dst,
                            in0=prod4[:, :, :dlen, :],
                            in1=prod4[:, :, off : off + dlen, :],
                        )
                    if not steps:  # GL == 1
                        nc.vector.tensor_copy(
                            out=ne3[:, :, D : 2 * D],
                            in_=prod4[:, :, 0, :],
                        )

                    # ne[..., 0:D] = fu * item
                    nc.vector.tensor_mul(
                        out=ne3[:, :, 0:D],
                        in0=ne3[:, :, D : 2 * D],
                        in1=ne3[:, :, 2 * D : 3 * D],
                    )

                    # MLP head: neT -> h = relu(W1 neT + b1) -> y
                    neT_ps = ppool.tile([3 * D, GP], bf, tag="neT",
                                        space="PSUM")
                    for j in range(g):
                        nc.tensor.transpose(
                            out=neT_ps[:, j * P : (j + 1) * P],
                            in_=ne3[:, j, :],
                            identity=id_sb[:],
                        )
                    neT_sb = wpool.tile([3 * D, GP], bf, tag="neTs")
                    nc.scalar.activation(
                        out=neT_sb[:],
                        in_=neT_ps[:],
                        func=mybir.ActivationFunctionType.Copy,
                    )

                    hT_ps = ppool.tile([8, GP], dt.float32, tag="hT",
                                       space="PSUM")
                    nc.tensor.matmul(
                        hT_ps[:],
                        lhsT=w1_sb[:],
                        rhs=neT_sb[:],
                        start=True,
                        stop=True,
                    )
                    hT_sb = wpool.tile([8, GP], bf, tag="hTs")
                    nc.scalar.activation(
                        out=hT_sb[:],
                        in_=hT_ps[:],
                        func=mybir.ActivationFunctionType.Relu,
                        bias=b1_sb[:, :1],
                    )

                    yT_ps = ppool.tile([1, GP], dt.float32, tag="yT",
                                       space="PSUM")
                    nc.tensor.matmul(
                        yT_ps[:],
                        lhsT=w2_sb[:],
                        rhs=hT_sb[:],
                        start=True,
                        stop=True,
                    )
                    y_sb = iopool.tile([1, GP], dt.float32, tag="y")
                    nc.scalar.activation(
                        out=y_sb[:],
                        in_=yT_ps[:],
                        func=mybir.ActivationFunctionType.Sigmoid,
                        bias=b2_sb[:1, :1],
                    )
                    nc.sync.dma_start(
                        out=y_out[gi * g : (gi + 1) * g, :], in_=y_sb[:]
                    )

    nc.compile()
    return nc


def _lengths_from_mask(mask_b):
    mm = np.asarray(mask_b, dtype=bool)
    pos = np.arange(1, M + 1, dtype=np.int32)
    return (mm * pos[None, :]).max(axis=1).astype(np.int32)


def prepare(item_inputs, member_ids, member_mask, n_cores=N_CORES):
    L = _lengths_from_mask(member_mask)
    order = np.argsort(-L, kind="stable")
    n = len(L)
    bc = n // n_cores
    nt = bc // P
    Ls = L[order]
    prof = [int(max(1, Ls[t * P * n_cores])) for t in range(nt)]
    return order, prof


def _wrap16(idv):
    """[n] int16 idx list -> [128, n/16] wrapped + replicated layout."""
    n = len(idv)
    w16 = idv.reshape(n // 16, 16).T
    return np.tile(w16, (8, 1))


def _make_in_maps(item_inputs, member_ids, member_mask, user_table, item_table,
                  W_bil, b_bil, W1, b1, W2, b2, order, prof, g=G):
    import ml_dtypes

    bf = ml_dtypes.bfloat16
    item_inputs = np.asarray(item_inputs).astype(np.int32).reshape(-1)
    member_ids = np.asarray(member_ids).astype(np.int32)
    mask_f = np.asarray(member_mask).astype(bf)
    user4 = np.ascontiguousarray(
        np.asarray(user_table, dtype=np.float32).astype(bf)
        .reshape(NU // 4, 4 * D)
    )
    item4 = np.ascontiguousarray(
        np.asarray(item_table, dtype=np.float32).astype(bf)
        .reshape(NI // 4, 4 * D)
    )
    w_bil_t = np.ascontiguousarray(np.asarray(W_bil, dtype=np.float32).T
                                   .astype(bf))
    w1_t = np.ascontiguousarray(np.asarray(W1, dtype=np.float32).T.astype(bf))
    w2_t = np.ascontiguousarray(np.asarray(W2, dtype=np.float32).T.astype(bf))
    b1_c = np.asarray(b1, dtype=np.float32).reshape(8, 1)
    b2_c = np.asarray(b2, dtype=np.float32).reshape(1, 1)
    bbil_c = np.full((P, 1), np.asarray(b_bil, dtype=np.float32).reshape(-1)[0],
                     dtype=np.float32)
    ident = np.eye(P, dtype=np.float32).astype(bf)

    gls = _group_gl(prof, g)
    ngroups = len(gls)

    in_maps = []
    for c in range(N_CORES):
        rows = order[c::N_CORES]
        mi = member_ids[rows]              # [bc, M]
        ii = item_inputs[rows]             # [bc]
        idx_parts, m23, modd, ip = [], [], [], []
        is23, isodd = [], []
        for gi in range(ngroups):
            GL = gls[gi]
            blk = mi[gi * g * P : (gi + 1) * g * P, :GL]     # [g*P, GL]
            b4 = blk.reshape(g, P, GL)
            idv = np.transpose(b4, (0, 2, 1)).reshape(-1)     # (j,m,p) order
            idx_parts.append(_wrap16((idv >> 2).astype(np.int16)))
            sub = (np.transpose(b4, (0, 2, 1)) & 3)           # [g, GL, P]
            subm = np.transpose(sub, (2, 0, 1)).reshape(P, g * GL)  # [p,(j,m)]
            m23.append((subm >= 2).astype(np.uint8))
            modd.append((subm & 1).astype(np.uint8))
            ib = ii[gi * g * P : (gi + 1) * g * P].reshape(g, P)
            iv = ib.reshape(-1)                                # (j,p) order
            ip.append(_wrap16((iv >> 2).astype(np.int16)))
            isub = (ib & 3).T                                  # [P, g]
            is23.append((isub >= 2).astype(np.uint8))
            isodd.append((isub & 1).astype(np.uint8))
        in_maps.append({
            "ids16": np.concatenate(idx_parts, axis=1),
            "iid16": np.concatenate(ip, axis=1),
            "msel23": np.concatenate(m23, axis=1),
            "mselodd": np.concatenate(modd, axis=1),
            "isel23": np.concatenate(is23, axis=1),
            "iselodd": np.concatenate(isodd, axis=1),
            "mask": np.ascontiguousarray(mask_f[rows]),
            "user4": user4,
            "item4": item4,
            "w_bil_t": w_bil_t,
            "w1_t": w1_t,
            "w2_t": w2_t,
            "b1": b1_c,
            "b2": b2_c,
            "bbil": bbil_c,
            "ident": ident,
        })
    return in_maps


def _get_compiled(prof):
    key = tuple(prof)
    if key not in _COMPILED:
        _COMPILED[key] = build_kernel(BC, G, prof=list(prof))
    return _COMPILED[key]


def run_on_hw(nc, in_maps, trace=False):
    from concourse import bass_utils

    return bass_utils.run_bass_kernel_spmd(
        nc, in_maps, core_ids=list(range(N_CORES)), trace=trace
    )


def kernel(item_inputs, member_ids, member_mask, user_table, item_table,
           W_bil, b_bil, W1, b1, W2, b2):
    order, prof = prepare(item_inputs, member_ids, member_mask)
    nc = _get_compiled(prof)
    in_maps = _make_in_maps(item_inputs, member_ids, member_mask, user_table,
                            item_table, W_bil, b_bil, W1, b1, W2, b2, order, prof)
    res = run_on_hw(nc, in_maps, trace=False)
    y = np.empty(B, dtype=np.float32)
    for c in range(N_CORES):
        y[order[c::N_CORES]] = res.results[c]["y"].reshape(BC)
    return y.reshape(B, 1)


# revision 45
# speedup vs baseline: 1.1989x; 1.1989x over previous
"""Trainium2 Bass kernel for nn_BILINEAR_56169582297414 (gnn message passing).

Reference computation (per prediction pair b):
    item_e = item_table[item_inputs[b]]                    # [D]
    mem_e  = user_table[member_ids[b, :]]                  # [M, D]
    scores[m] = mem_e[m] @ W_bil @ item_e + b_bil          # bilinear
    w = scores * member_mask[b]                            # mask padded members
    fu = sum_m w[m] * mem_e[m]                             # [D]
    ne = [fu * item_e, fu, item_e]                         # [3D]
    y = sigmoid(relu(ne @ W1.T + b1) @ W2.T + b2)          # [1]

Strategy: data-parallel over 8 NeuronCores, tables replicated. The serial
resource is SWDGE descriptor generation on the GPSIMD (Pool) engine; each
dma_gather runs on ONE Q7 core-pair selected by queue_num, so gathers for 4
consecutive tiles are issued back-to-back on queues 0-3 to run on all 4
core-pairs concurrently.  Rows are sorted by true group length (desc) and
striped across cores so each tile fetches only maxL members.  Tables are
bf16 4-packed (256B gather elements, idx = id>>2 fits int16); the 1-of-4
sub-row select uses DVE copy_predicated with host-precomputed masks.
Scores/weighted-sum run on DVE in bf16 with a contiguous halving tree for
the member reduction; bilinear projection + MLP head run on TensorE with
batched (per-4-tile) matmuls.
"""

import sys

sys.path.insert(0, "/opt/trn_rl_repo")

import numpy as np

B = 262144
M = 16
NU = 100000
NI = 50000
D = 32
N_CORES = 8
BC = B // N_CORES
P = 128
NT = BC // P
G = 4  # tiles per group: one member-gather instruction per group, 4 queues

_COMPILED = {}


def _group_gl(prof, g=G):
    """Per-group max member count (prof is non-increasing)."""
    return [max(prof[i * g : (i + 1) * g]) for i in range(len(prof) // g)]


def _tree_steps(gl):
    """Halving-tree add steps for ragged length gl -> [(dst_len, src_off)]."""
    steps = []
    n = gl
    while n > 1:
        h = (n + 1) // 2
        steps.append((n - h, h))  # add cols [h, n) onto [0, n-h)
        n = h
    return steps


def build_kernel(bc, g=G, prof=None):
    """Per-core Bass program against bf16 4-packed tables user4 [25000, 128]
    and item4 [12500, 128]."""
    import concourse.bacc as bacc
    import concourse.tile as tile
    from concourse import mybir
    from concourse.library_config import mlp

    nt = bc // P
    assert nt % g == 0
    ngroups = nt // g
    if prof is None:
        prof = [M] * nt
    prof = [int(max(1, min(M, x))) for x in prof]
    gls = _group_gl(prof, g)
    dt = mybir.dt
    bf = dt.bfloat16

    # flat col offsets for per-group idx/mask tensors
    idx_cols = [g * gl * 8 for gl in gls]       # int16 cols ([128, .])
    jm_cols = [g * gl for gl in gls]            # member-mask cols
    idx_off = np.concatenate([[0], np.cumsum(idx_cols)]).astype(int)
    jm_off = np.concatenate([[0], np.cumsum(jm_cols)]).astype(int)

    nc = bacc.Bacc(
        "TRN2", target_bir_lowering=False, debug=False, num_swdge_queues=4
    )

    ids16 = nc.dram_tensor("ids16", [P, int(idx_off[-1])], dt.int16,
                           kind="ExternalInput")
    iid16 = nc.dram_tensor("iid16", [P, ngroups * g * 8], dt.int16,
                           kind="ExternalInput")
    msel = [
        nc.dram_tensor(f"msel{q}", [P, int(jm_off[-1])], dt.uint8,
                       kind="ExternalInput")
        for q in ("23", "odd")
    ]
    isel = [
        nc.dram_tensor(f"isel{q}", [P, ngroups * g], dt.uint8,
                       kind="ExternalInput")
        for q in ("23", "odd")
    ]
    mask = nc.dram_tensor("mask", [bc, M], bf, kind="ExternalInput")
    user4 = nc.dram_tensor("user4", [NU // 4, 4 * D], bf, kind="ExternalInput")
    item4 = nc.dram_tensor("item4", [NI // 4, 4 * D], bf, kind="ExternalInput")
    w_bil_t = nc.dram_tensor("w_bil_t", [D, D], bf, kind="ExternalInput")
    w1_t = nc.dram_tensor("w1_t", [3 * D, 8], bf, kind="ExternalInput")
    w2_t = nc.dram_tensor("w2_t", [8, 1], bf, kind="ExternalInput")
    b1 = nc.dram_tensor("b1", [8, 1], dt.float32, kind="ExternalInput")
    b2 = nc.dram_tensor("b2", [1, 1], dt.float32, kind="ExternalInput")
    bbil = nc.dram_tensor("bbil", [P, 1], dt.float32, kind="ExternalInput")
    ident = nc.dram_tensor("ident", [P, P], bf, kind="ExternalInput")
    y_out = nc.dram_tensor("y", [nt, P], dt.float32, kind="ExternalOutput")

    GM = g * M
    GP = g * P
    SB = 4  # groups per issue batch (one per SWDGE queue)
    assert ngroups % SB == 0
    nbatch = ngroups // SB

    with tile.TileContext(nc) as tc:
        with (
            tc.tile_pool(name="const", bufs=1) as cpool,
            tc.tile_pool(name="io", bufs=4) as iopool,
            tc.tile_pool(name="work", bufs=3) as wpool,
            tc.tile_pool(name="gath", bufs=2) as gpool,
            tc.tile_pool(name="gath2", bufs=2) as g2pool,
            tc.tile_pool(name="prodp", bufs=2) as prpool,
            tc.tile_pool(name="memp", bufs=4) as mpool,
            tc.tile_pool(name="psum", bufs=1, space="PSUM") as ppool,
            tc.tile_pool(name="psumv", bufs=2, space="PSUM") as ppoolv,
        ):
            with tc.tile_critical():
                nc.gpsimd.load_library(mlp)

            wt_sb = cpool.tile([D, D], bf, tag="wt")
            nc.sync.dma_start(out=wt_sb[:], in_=w_bil_t[:])
            w1_sb = cpool.tile([3 * D, 8], bf, tag="w1")
            nc.sync.dma_start(out=w1_sb[:], in_=w1_t[:])
            w2_sb = cpool.tile([8, 1], bf, tag="w2")
            nc.sync.dma_start(out=w2_sb[:], in_=w2_t[:])
            b1_sb = cpool.tile([8, 1], dt.float32, tag="b1")
            nc.sync.dma_start(out=b1_sb[:], in_=b1[:])
            b2_sb = cpool.tile([1, 1], dt.float32, tag="b2")
            nc.sync.dma_start(out=b2_sb[:], in_=b2[:])
            bbil_sb = cpool.tile([P, 1], dt.float32, tag="bbil")
            nc.sync.dma_start(out=bbil_sb[:], in_=bbil[:])
            id_sb = cpool.tile([P, P], bf, tag="ident")
            nc.sync.dma_start(out=id_sb[:], in_=ident[:])

            def load_batch(bj):
                """Issue batched input DMAs for batch bj; returns tile dict."""
                h0 = bj * SB
                i0, i1 = int(idx_off[h0]), int(idx_off[h0 + SB])
                j0, j1 = int(jm_off[h0]), int(jm_off[h0 + SB])
                ids_b = iopool.tile([P, i1 - i0], dt.int16, tag="ids")
                nc.sync.dma_start(out=ids_b[:], in_=ids16[:, i0:i1])
                ms_b = []
                for s in range(2):
                    m_sb = iopool.tile([P, j1 - j0], dt.uint8, tag=f"ms{s}")
                    nc.sync.dma_start(out=m_sb[:], in_=msel[s][:, j0:j1])
                    ms_b.append(m_sb)
                mask_b = iopool.tile([P, SB * GM], bf, tag="mask")
                nc.sync.dma_start(
                    out=mask_b[:].rearrange("p (q g m) -> p q g m", q=SB, g=g),
                    in_=mask[h0 * GP : (h0 + SB) * GP, :]
                        .rearrange("(q g p) m -> p q g m", p=P, g=g),
                )
                iid_b = iopool.tile([P, SB * g * 8], dt.int16, tag="iid")
                nc.sync.dma_start(
                    out=iid_b[:],
                    in_=iid16[:, h0 * g * 8 : (h0 + SB) * g * 8],
                )
                is_b = []
                for s in range(2):
                    i_sb = iopool.tile([P, SB * g], dt.uint8, tag=f"is{s}")
                    nc.sync.dma_start(
                        out=i_sb[:], in_=isel[s][:, h0 * g : (h0 + SB) * g]
                    )
                    is_b.append(i_sb)
                return {"ids": ids_b, "ms": ms_b, "mask": mask_b,
                        "iid": iid_b, "is": is_b, "i0": i0, "j0": j0}

            LA = 2  # input-load lookahead (batches)
            loaded = {}
            for bj in range(min(LA + 1, nbatch)):
                loaded[bj] = load_batch(bj)

            for bi in range(nbatch):
                g0 = bi * SB
                bgls = gls[g0 : g0 + SB]
                if bi + LA + 1 < nbatch:
                    loaded[bi + LA + 1] = load_batch(bi + LA + 1)
                bt = loaded.pop(bi)
                iid_sb = bt["iid"]
                is_sbs = bt["is"]

                # --- one batched item gather first (small, 16 tiles) -----
                g2_sb = g2pool.tile([P, SB * g * 4 * D], bf, tag="g2")
                g2 = g2_sb[:].rearrange("p (c e) -> p c e", c=SB * g)
                nc.gpsimd.dma_gather(
                    out_ap=g2,
                    in_ap=item4[:],
                    idxs_ap=iid_sb[:],
                    num_idxs=SB * g * 128,
                    num_idxs_reg=SB * g * 128,
                    elem_size=4 * D,
                    single_packet=False,
                    queue_num=bi % 4,
                )

                # --- 4 member gathers, queues 0-3, back to back ----------
                g4s = []
                for q in range(SB):
                    gi = g0 + q
                    GL = bgls[q]
                    nmi = g * GL * 128
                    ids_ap = bt["ids"][
                        :, int(idx_off[gi]) - bt["i0"]
                           : int(idx_off[gi + 1]) - bt["i0"]
                    ]
                    g4_sb = gpool.tile([P, g * GL * 4 * D], bf, tag=f"g4_{q}")
                    g4 = g4_sb[:].rearrange("p (c e) -> p c e", c=g * GL)
                    nc.gpsimd.dma_gather(
                        out_ap=g4,
                        in_ap=user4[:],
                        idxs_ap=ids_ap,
                        num_idxs=nmi,
                        num_idxs_reg=nmi,
                        elem_size=4 * D,
                        single_packet=False,
                        queue_num=q,
                    )
                    g4s.append(g4)

                # --- select phase: all 4 groups' member selects first, so
                # the raw g4 buffers free a full batch earlier ------------
                mem_sbs = []
                for q in range(SB):
                    gi = g0 + q
                    GL = bgls[q]
                    C = g * GL
                    # 1-of-4 member sub-row select, in place in g4 via a
                    # binary tree: pick the 2D pair by (sub>=2), then the
                    # D row by (sub&1).  mem stays at g4[:, :, 0:D].
                    g4 = g4s[q]
                    jlo = int(jm_off[gi]) - bt["j0"]
                    jhi = int(jm_off[gi + 1]) - bt["j0"]
                    m23_ap = bt["ms"][0][:, jlo:jhi]
                    modd_ap = bt["ms"][1][:, jlo:jhi]
                    nc.vector.copy_predicated(
                        out=g4[:, :, 0 : 2 * D],
                        mask=m23_ap.unsqueeze(2).broadcast_to([P, C, 2 * D]),
                        data=g4[:, :, 2 * D : 4 * D],
                    )
                    nc.vector.copy_predicated(
                        out=g4[:, :, 0:D],
                        mask=modd_ap.unsqueeze(2).broadcast_to([P, C, D]),
                        data=g4[:, :, D : 2 * D],
                    )
                    # compact selected rows out of g4 on the Scalar engine:
                    # frees the raw gather buffer early and gives the DVE
                    # multiplies contiguous reads
                    mem_sb = mpool.tile([P, C * D], bf, tag="mem")
                    nc.scalar.activation(
                        out=mem_sb[:].rearrange("p (c d) -> p c d", c=C),
                        in_=g4[:, :, 0:D],
                        func=mybir.ActivationFunctionType.Copy,
                    )
                    mem_sbs.append(mem_sb)

                # --- math phase ------------------------------------------
                for q in range(SB):
                    gi = g0 + q
                    GL = bgls[q]
                    C = g * GL
                    mem_sb = mem_sbs[q]

                    # item 1-of-4 select, in place in g2; scalar engine
                    # copies the result into ne[..., 2D:3D]
                    ne_sb = wpool.tile([P, g * 3 * D], bf, tag="ne")
                    ne3 = ne_sb[:].rearrange("p (g c) -> p g c", g=g)
                    g2q = g2[:, q * g : (q + 1) * g, :]
                    nc.vector.copy_predicated(
                        out=g2q[:, :, 0 : 2 * D],
                        mask=is_sbs[0][:, q * g : (q + 1) * g]
                             .unsqueeze(2).broadcast_to([P, g, 2 * D]),
                        data=g2q[:, :, 2 * D : 4 * D],
                    )
                    nc.vector.copy_predicated(
                        out=g2q[:, :, 0:D],
                        mask=is_sbs[1][:, q * g : (q + 1) * g]
                             .unsqueeze(2).broadcast_to([P, g, D]),
                        data=g2q[:, :, D : 2 * D],
                    )
                    nc.scalar.activation(
                        out=ne3[:, :, 2 * D : 3 * D],
                        in_=g2q[:, :, 0:D],
                        func=mybir.ActivationFunctionType.Copy,
                    )

                    # itemT via PE transpose (bf16 PSUM), then v = W^T @ item
                    itemT_ps = ppool.tile([D, GP], bf, tag="itemT",
                                          space="PSUM")
                    for j in range(g):
                        nc.tensor.transpose(
                            out=itemT_ps[:, j * P : (j + 1) * P],
                            in_=g2q[:, j, 0:D],
                            identity=id_sb[:],
                        )
                    itemT_sb = wpool.tile([D, GP], bf, tag="itemT")
                    nc.scalar.activation(
                        out=itemT_sb[:],
                        in_=itemT_ps[:],
                        func=mybir.ActivationFunctionType.Copy,
                    )

                    v_ps = ppoolv.tile([P, g * D], dt.float32, tag="v",
                                       space="PSUM")
                    for j in range(g):
                        nc.tensor.matmul(
                            v_ps[:, j * D : (j + 1) * D],
                            lhsT=itemT_sb[:, j * P : (j + 1) * P],
                            rhs=wt_sb[:],
                            start=True,
                            stop=True,
                        )
                    v_sb = wpool.tile([P, g * D], bf, tag="vsb")
                    nc.scalar.activation(
                        out=v_sb[:],
                        in_=v_ps[:],
                        func=mybir.ActivationFunctionType.Copy,
                    )

                    # scores = sum_d mem * v  (X-reduce over d)
                    mem4 = mem_sb[:].rearrange("p (g m d) -> p g m d",
                                               g=g, m=GL)
                    v_b = (
                        v_sb[:]
                        .rearrange("p (g d) -> p g d", g=g)
                        .unsqueeze(2)
                        .broadcast_to([P, g, GL, D])
                    )
                    prod_sb = prpool.tile([P, GM * D], bf, tag="prod")
                    prod4 = prod_sb[:].rearrange("p (g m d) -> p g m d",
                                                 g=g, m=M)[:, :, :GL, :]
                    nc.vector.tensor_mul(out=prod4, in0=mem4, in1=v_b)

                    scores_sb = wpool.tile([P, GM], dt.float32, tag="scores")
                    sc3 = scores_sb[:].rearrange("p (g m) -> p g m", g=g)
                    nc.vector.reduce_sum(
                        out=sc3[:, :, :GL], in_=prod4,
                        axis=mybir.AxisListType.X,
                    )

                    # w = (scores + b_bil) * mask
                    w_sb = wpool.tile([P, GM], bf, tag="w")
                    w3 = w_sb[:].rearrange("p (g m) -> p g m", g=g)
                    m3 = bt["mask"][:].rearrange(
                        "p (q g m) -> p q g m", q=SB, g=g
                    )[:, q]
                    nc.vector.scalar_tensor_tensor(
                        out=w3[:, :, :GL],
                        in0=sc3[:, :, :GL],
                        scalar=bbil_sb[:, :1],
                        in1=m3[:, :, :GL],
                        op0=mybir.AluOpType.add,
                        op1=mybir.AluOpType.mult,
                    )

                    # prod2 = w * mem, then halving-tree sum over m -> fu
                    w_b = w3[:, :, :GL].unsqueeze(3).broadcast_to([P, g, GL, D])
                    nc.vector.tensor_mul(out=prod4, in0=mem4, in1=w_b)

                    steps = _tree_steps(GL)
                    prod4 = prod_sb[:].rearrange("p (g m d) -> p g m d",
                                                 g=g, m=M)[:, :, :GL, :]
                    for si, (dlen, off) in enumerate(steps):
                        last = si == len(steps) - 1
                        dst = (ne3[:, :, D : 2 * D].unsqueeze(2)
                               if last and dlen == 1
                               else prod4[:, :, :dlen, :])
                        nc.vector.tensor_add(
                            out=dst,
                            in0=prod4[:, :, :dlen, :],
                            in1=prod4[:, :, off : off + dlen, :],
                        )
                    if not steps:  # GL == 1
                        nc.vector.tensor_copy(
                            out=ne3[:, :, D : 2 * D],
                            in_=prod4[:, :, 0, :],
                        )

                    # ne[..., 0:D] = fu * item
                    nc.vector.tensor_mul(
                        out=ne3[:, :, 0:D],
                        in0=ne3[:, :, D : 2 * D],
                        in1=ne3[:, :, 2 * D : 3 * D],
                    )

                    # MLP head: neT -> h = relu(W1 neT + b1) -> y
                    neT_ps = ppool.tile([3 * D, GP], bf, tag="neT",
                                        space="PSUM")
                    for j in range(g):
                        nc.tensor.transpose(
                            out=neT_ps[:, j * P : (j + 1) * P],
                            in_=ne3[:, j, :],
                            identity=id_sb[:],
                        )
                    neT_sb = wpool.tile([3 * D, GP], bf, tag="neTs")
                    nc.scalar.activation(
                        out=neT_sb[:],
                        in_=neT_ps[:],
                        func=mybir.ActivationFunctionType.Copy,
                    )

                    hT_ps = ppool.tile([8, GP], dt.float32, tag="hT",
                                       space="PSUM")
                    nc.tensor.matmul(
                        hT_ps[:],
                        lhsT=w1_sb[:],
                        rhs=neT_sb[:],
                        start=True,
                        stop=True,
                    )
                    hT_sb = wpool.tile([8, GP], bf, tag="hTs")
                    nc.scalar.activation(
                        out=hT_sb[:],
                        in_=hT_ps[:],
                        func=mybir.ActivationFunctionType.Relu,
                        bias=b1_sb[:, :1],
                    )

                    yT_ps = ppool.tile([1, GP], dt.float32, tag="yT",
                                       space="PSUM")
                    nc.tensor.matmul(
                        yT_ps[:],
                        lhsT=w2_sb[:],
                        rhs=hT_sb[:],
                        start=True,
                        stop=True,
                    )
                    y_sb = iopool.tile([1, GP], dt.float32, tag="y")
                    nc.scalar.activation(
                        out=y_sb[:],
                        in_=yT_ps[:],
                        func=mybir.ActivationFunctionType.Sigmoid,
                        bias=b2_sb[:1, :1],
                    )
                    nc.sync.dma_start(
                        out=y_out[gi * g : (gi + 1) * g, :], in_=y_sb[:]
                    )

    nc.compile()
    return nc


def _lengths_from_mask(mask_b):
    mm = np.asarray(mask_b, dtype=bool)
    pos = np.arange(1, M + 1, dtype=np.int32)
    return (mm * pos[None, :]).max(axis=1).astype(np.int32)


def prepare(item_inputs, member_ids, member_mask, n_cores=N_CORES):
    L = _lengths_from_mask(member_mask)
    order = np.argsort(-L, kind="stable")
    n = len(L)
    bc = n // n_cores
    nt = bc // P
    Ls = L[order]
    prof = [int(max(1, Ls[t * P * n_cores])) for t in range(nt)]
    return order, prof


def _wrap16(idv):
    """[n] int16 idx list -> [128, n/16] wrapped + replicated layout."""
    n = len(idv)
    w16 = idv.reshape(n // 16, 16).T
    return np.tile(w16, (8, 1))


def _make_in_maps(item_inputs, member_ids, member_mask, user_table, item_table,
                  W_bil, b_bil, W1, b1, W2, b2, order, prof, g=G):
    import ml_dtypes

    bf = ml_dtypes.bfloat16
    item_inputs = np.asarray(item_inputs).astype(np.int32).reshape(-1)
    member_ids = np.asarray(member_ids).astype(np.int32)
    mask_f = np.asarray(member_mask).astype(bf)
    user4 = np.ascontiguousarray(
        np.asarray(user_table, dtype=np.float32).astype(bf)
        .reshape(NU // 4, 4 * D)
    )
    item4 = np.ascontiguousarray(
        np.asarray(item_table, dtype=np.float32).astype(bf)
        .reshape(NI // 4, 4 * D)
    )
    w_bil_t = np.ascontiguousarray(np.asarray(W_bil, dtype=np.float32).T
                                   .astype(bf))
    w1_t = np.ascontiguousarray(np.asarray(W1, dtype=np.float32).T.astype(bf))
    w2_t = np.ascontiguousarray(np.asarray(W2, dtype=np.float32).T.astype(bf))
    b1_c = np.asarray(b1, dtype=np.float32).reshape(8, 1)
    b2_c = np.asarray(b2, dtype=np.float32).reshape(1, 1)
    bbil_c = np.full((P, 1), np.asarray(b_bil, dtype=np.float32).reshape(-1)[0],
                     dtype=np.float32)
    ident = np.eye(P, dtype=np.float32).astype(bf)

    gls = _group_gl(prof, g)
    ngroups = len(gls)

    in_maps = []
    for c in range(N_CORES):
        rows = order[c::N_CORES]
        mi = member_ids[rows]              # [bc, M]
        ii = item_inputs[rows]             # [bc]
        idx_parts, m23, modd, ip = [], [], [], []
        is23, isodd = [], []
        for gi in range(ngroups):
            GL = gls[gi]
            blk = mi[gi * g * P : (gi + 1) * g * P, :GL]     # [g*P, GL]
            b4 = blk.reshape(g, P, GL)
            idv = np.transpose(b4, (0, 2, 1)).reshape(-1)     # (j,m,p) order
            idx_parts.append(_wrap16((idv >> 2).astype(np.int16)))
            sub = (np.transpose(b4, (0, 2, 1)) & 3)           # [g, GL, P]
            subm = np.transpose(sub, (2, 0, 1)).reshape(P, g * GL)  # [p,(j,m)]
            m23.append((subm >= 2).astype(np.uint8))
            modd.append((subm & 1).astype(np.uint8))
            ib = ii[gi * g * P : (gi + 1) * g * P].reshape(g, P)
            iv = ib.reshape(-1)                                # (j,p) order
            ip.append(_wrap16((iv >> 2).astype(np.int16)))
            isub = (ib & 3).T                                  # [P, g]
            is23.append((isub >= 2).astype(np.uint8))
            isodd.append((isub & 1).astype(np.uint8))
        in_maps.append({
            "ids16": np.concatenate(idx_parts, axis=1),
            "iid16": np.concatenate(ip, axis=1),
            "msel23": np.concatenate(m23, axis=1),
            "mselodd": np.concatenate(modd, axis=1),
            "isel23": np.concatenate(is23, axis=1),
            "iselodd": np.concatenate(isodd, axis=1),
            "mask": np.ascontiguousarray(mask_f[rows]),
            "user4": user4,
            "item4": item4,
            "w_bil_t": w_bil_t,
            "w1_t": w1_t,
            "w2_t": w2_t,
            "b1": b1_c,
            "b2": b2_c,
            "bbil": bbil_c,
            "ident": ident,
        })
    return in_maps


def _get_compiled(prof):
    key = tuple(prof)
    if key not in _COMPILED:
        _COMPILED[key] = build_kernel(BC, G, prof=list(prof))
    return _COMPILED[key]


def run_on_hw(nc, in_maps, trace=False):
    from concourse import bass_utils

    return bass_utils.run_bass_kernel_spmd(
        nc, in_maps, core_ids=list(range(N_CORES)), trace=trace
    )


def kernel(item_inputs, member_ids, member_mask, user_table, item_table,
           W_bil, b_bil, W1, b1, W2, b2):
    order, prof = prepare(item_inputs, member_ids, member_mask)
    nc = _get_compiled(prof)
    in_maps = _make_in_maps(item_inputs, member_ids, member_mask, user_table,
                            item_table, W_bil, b_bil, W1, b1, W2, b2, order, prof)
    res = run_on_hw(nc, in_maps, trace=False)
    y = np.empty(B, dtype=np.float32)
    for c in range(N_CORES):
        y[order[c::N_CORES]] = res.results[c]["y"].reshape(BC)
    return y.reshape(B, 1)


# revision 48
# speedup vs baseline: 1.2062x; 1.0060x over previous
"""Trainium2 Bass kernel for nn_BILINEAR_56169582297414 (gnn message passing).

Reference computation (per prediction pair b):
    item_e = item_table[item_inputs[b]]                    # [D]
    mem_e  = user_table[member_ids[b, :]]                  # [M, D]
    scores[m] = mem_e[m] @ W_bil @ item_e + b_bil          # bilinear
    w = scores * member_mask[b]                            # mask padded members
    fu = sum_m w[m] * mem_e[m]                             # [D]
    ne = [fu * item_e, fu, item_e]                         # [3D]
    y = sigmoid(relu(ne @ W1.T + b1) @ W2.T + b2)          # [1]

Strategy: data-parallel over 8 NeuronCores, tables replicated. The serial
resource is SWDGE descriptor generation on the GPSIMD (Pool) engine; each
dma_gather runs on ONE Q7 core-pair selected by queue_num, so gathers for 4
consecutive tiles are issued back-to-back on queues 0-3 to run on all 4
core-pairs concurrently.  Rows are sorted by true group length (desc) and
striped across cores so each tile fetches only maxL members.  Tables are
bf16 4-packed (256B gather elements, idx = id>>2 fits int16); the 1-of-4
sub-row select uses DVE copy_predicated with host-precomputed masks.
Scores/weighted-sum run on DVE in bf16 with a contiguous halving tree for
the member reduction; bilinear projection + MLP head run on TensorE with
batched (per-4-tile) matmuls.
"""

import sys

sys.path.insert(0, "/opt/trn_rl_repo")

import numpy as np

B = 262144
M = 16
NU = 100000
NI = 50000
D = 32
N_CORES = 8
BC = B // N_CORES
P = 128
NT = BC // P
G = 4  # tiles per group: one member-gather instruction per group, 4 queues

_COMPILED = {}


def _group_gl(prof, g=G):
    """Per-group max member count (prof is non-increasing)."""
    return [max(prof[i * g : (i + 1) * g]) for i in range(len(prof) // g)]


def _tree_steps(gl):
    """Halving-tree add steps for ragged length gl -> [(dst_len, src_off)]."""
    steps = []
    n = gl
    while n > 1:
        h = (n + 1) // 2
        steps.append((n - h, h))  # add cols [h, n) onto [0, n-h)
        n = h
    return steps


def build_kernel(bc, g=G, prof=None):
    """Per-core Bass program against bf16 4-packed tables user4 [25000, 128]
    and item4 [12500, 128]."""
    import concourse.bacc as bacc
    import concourse.tile as tile
    from concourse import mybir
    from concourse.library_config import mlp

    nt = bc // P
    assert nt % g == 0
    ngroups = nt // g
    if prof is None:
        prof = [M] * nt
    prof = [int(max(1, min(M, x))) for x in prof]
    gls = _group_gl(prof, g)
    dt = mybir.dt
    bf = dt.bfloat16

    # flat col offsets for per-group idx/mask tensors
    idx_cols = [g * gl * 8 for gl in gls]       # int16 cols ([128, .])
    jm_cols = [g * gl for gl in gls]            # member-mask cols
    idx_off = np.concatenate([[0], np.cumsum(idx_cols)]).astype(int)
    jm_off = np.concatenate([[0], np.cumsum(jm_cols)]).astype(int)

    nc = bacc.Bacc(
        "TRN2", target_bir_lowering=False, debug=False, num_swdge_queues=4
    )

    ids16 = nc.dram_tensor("ids16", [P, int(idx_off[-1])], dt.int16,
                           kind="ExternalInput")
    iid16 = nc.dram_tensor("iid16", [P, ngroups * g * 8], dt.int16,
                           kind="ExternalInput")
    msel = [
        nc.dram_tensor(f"msel{q}", [P, int(jm_off[-1])], dt.uint8,
                       kind="ExternalInput")
        for q in ("23", "odd")
    ]
    isel = [
        nc.dram_tensor(f"isel{q}", [P, ngroups * g], dt.uint8,
                       kind="ExternalInput")
        for q in ("23", "odd")
    ]
    mask = nc.dram_tensor("mask", [bc, M], bf, kind="ExternalInput")
    user4 = nc.dram_tensor("user4", [NU // 4, 4 * D], bf, kind="ExternalInput")
    item4 = nc.dram_tensor("item4", [NI // 4, 4 * D], bf, kind="ExternalInput")
    w_bil_t = nc.dram_tensor("w_bil_t", [D, D], bf, kind="ExternalInput")
    w1_t = nc.dram_tensor("w1_t", [3 * D, 8], bf, kind="ExternalInput")
    w2_t = nc.dram_tensor("w2_t", [8, 1], bf, kind="ExternalInput")
    b1 = nc.dram_tensor("b1", [8, 1], dt.float32, kind="ExternalInput")
    b2 = nc.dram_tensor("b2", [1, 1], dt.float32, kind="ExternalInput")
    bbil = nc.dram_tensor("bbil", [P, 1], dt.float32, kind="ExternalInput")
    ident = nc.dram_tensor("ident", [P, P], bf, kind="ExternalInput")
    y_out = nc.dram_tensor("y", [nt, P], dt.float32, kind="ExternalOutput")

    GM = g * M
    GP = g * P
    SB = 4  # groups per issue batch (one per SWDGE queue)
    assert ngroups % SB == 0
    nbatch = ngroups // SB

    with tile.TileContext(nc) as tc:
        with (
            tc.tile_pool(name="const", bufs=1) as cpool,
            tc.tile_pool(name="io", bufs=4) as iopool,
            tc.tile_pool(name="work", bufs=3) as wpool,
            tc.tile_pool(name="gath", bufs=2) as gpool,
            tc.tile_pool(name="gath2", bufs=2) as g2pool,
            tc.tile_pool(name="prodp", bufs=2) as prpool,
            tc.tile_pool(name="memp", bufs=4) as mpool,
            tc.tile_pool(name="nep", bufs=5) as nepool,
            tc.tile_pool(name="psum", bufs=1, space="PSUM") as ppool,
            tc.tile_pool(name="psumv", bufs=2, space="PSUM") as ppoolv,
        ):
            with tc.tile_critical():
                nc.gpsimd.load_library(mlp)

            wt_sb = cpool.tile([D, D], bf, tag="wt")
            nc.sync.dma_start(out=wt_sb[:], in_=w_bil_t[:])
            w1_sb = cpool.tile([3 * D, 8], bf, tag="w1")
            nc.sync.dma_start(out=w1_sb[:], in_=w1_t[:])
            w2_sb = cpool.tile([8, 1], bf, tag="w2")
            nc.sync.dma_start(out=w2_sb[:], in_=w2_t[:])
            b1_sb = cpool.tile([8, 1], dt.float32, tag="b1")
            nc.sync.dma_start(out=b1_sb[:], in_=b1[:])
            b2_sb = cpool.tile([1, 1], dt.float32, tag="b2")
            nc.sync.dma_start(out=b2_sb[:], in_=b2[:])
            bbil_sb = cpool.tile([P, 1], dt.float32, tag="bbil")
            nc.sync.dma_start(out=bbil_sb[:], in_=bbil[:])
            id_sb = cpool.tile([P, P], bf, tag="ident")
            nc.sync.dma_start(out=id_sb[:], in_=ident[:])

            def load_batch(bj):
                """Issue batched input DMAs for batch bj; returns tile dict."""
                h0 = bj * SB
                i0, i1 = int(idx_off[h0]), int(idx_off[h0 + SB])
                j0, j1 = int(jm_off[h0]), int(jm_off[h0 + SB])
                ids_b = iopool.tile([P, i1 - i0], dt.int16, tag="ids")
                nc.sync.dma_start(out=ids_b[:], in_=ids16[:, i0:i1])
                ms_b = []
                for s in range(2):
                    m_sb = iopool.tile([P, j1 - j0], dt.uint8, tag=f"ms{s}")
                    nc.sync.dma_start(out=m_sb[:], in_=msel[s][:, j0:j1])
                    ms_b.append(m_sb)
                mask_b = iopool.tile([P, SB * GM], bf, tag="mask")
                nc.sync.dma_start(
                    out=mask_b[:].rearrange("p (q g m) -> p q g m", q=SB, g=g),
                    in_=mask[h0 * GP : (h0 + SB) * GP, :]
                        .rearrange("(q g p) m -> p q g m", p=P, g=g),
                )
                iid_b = iopool.tile([P, SB * g * 8], dt.int16, tag="iid")
                nc.sync.dma_start(
                    out=iid_b[:],
                    in_=iid16[:, h0 * g * 8 : (h0 + SB) * g * 8],
                )
                is_b = []
                for s in range(2):
                    i_sb = iopool.tile([P, SB * g], dt.uint8, tag=f"is{s}")
                    nc.sync.dma_start(
                        out=i_sb[:], in_=isel[s][:, h0 * g : (h0 + SB) * g]
                    )
                    is_b.append(i_sb)
                return {"ids": ids_b, "ms": ms_b, "mask": mask_b,
                        "iid": iid_b, "is": is_b, "i0": i0, "j0": j0}

            LA = 2  # input-load lookahead (batches)
            loaded = {}
            for bj in range(min(LA + 1, nbatch)):
                loaded[bj] = load_batch(bj)

            for bi in range(nbatch):
                g0 = bi * SB
                bgls = gls[g0 : g0 + SB]
                if bi + LA + 1 < nbatch:
                    loaded[bi + LA + 1] = load_batch(bi + LA + 1)
                bt = loaded.pop(bi)
                iid_sb = bt["iid"]
                is_sbs = bt["is"]

                # --- one batched item gather first (small, 16 tiles) -----
                g2_sb = g2pool.tile([P, SB * g * 4 * D], bf, tag="g2")
                g2 = g2_sb[:].rearrange("p (c e) -> p c e", c=SB * g)
                nc.gpsimd.dma_gather(
                    out_ap=g2,
                    in_ap=item4[:],
                    idxs_ap=iid_sb[:],
                    num_idxs=SB * g * 128,
                    num_idxs_reg=SB * g * 128,
                    elem_size=4 * D,
                    single_packet=False,
                    queue_num=bi % 4,
                )

                # --- 4 member gathers, queues 0-3, back to back ----------
                g4s = []
                for q in range(SB):
                    gi = g0 + q
                    GL = bgls[q]
                    nmi = g * GL * 128
                    ids_ap = bt["ids"][
                        :, int(idx_off[gi]) - bt["i0"]
                           : int(idx_off[gi + 1]) - bt["i0"]
                    ]
                    g4_sb = gpool.tile([P, g * GL * 4 * D], bf, tag=f"g4_{q}")
                    g4 = g4_sb[:].rearrange("p (c e) -> p c e", c=g * GL)
                    nc.gpsimd.dma_gather(
                        out_ap=g4,
                        in_ap=user4[:],
                        idxs_ap=ids_ap,
                        num_idxs=nmi,
                        num_idxs_reg=nmi,
                        elem_size=4 * D,
                        single_packet=False,
                        queue_num=q,
                    )
                    g4s.append(g4)

                # --- select phase: all 4 groups' member selects first, so
                # the raw g4 buffers free a full batch earlier ------------
                mem_sbs, ne_sbs = [], []
                for q in range(SB):
                    gi = g0 + q
                    GL = bgls[q]
                    C = g * GL
                    # 1-of-4 member sub-row select, in place in g4 via a
                    # binary tree: pick the 2D pair by (sub>=2), then the
                    # D row by (sub&1).  mem stays at g4[:, :, 0:D].
                    g4 = g4s[q]
                    jlo = int(jm_off[gi]) - bt["j0"]
                    jhi = int(jm_off[gi + 1]) - bt["j0"]
                    m23_ap = bt["ms"][0][:, jlo:jhi]
                    modd_ap = bt["ms"][1][:, jlo:jhi]
                    nc.vector.copy_predicated(
                        out=g4[:, :, 0 : 2 * D],
                        mask=m23_ap.unsqueeze(2).broadcast_to([P, C, 2 * D]),
                        data=g4[:, :, 2 * D : 4 * D],
                    )
                    nc.vector.copy_predicated(
                        out=g4[:, :, 0:D],
                        mask=modd_ap.unsqueeze(2).broadcast_to([P, C, D]),
                        data=g4[:, :, D : 2 * D],
                    )
                    # compact selected rows out of g4 on the Scalar engine:
                    # frees the raw gather buffer early and gives the DVE
                    # multiplies contiguous reads
                    mem_sb = mpool.tile([P, C * D], bf, tag="mem")
                    nc.scalar.activation(
                        out=mem_sb[:].rearrange("p (c d) -> p c d", c=C),
                        in_=g4[:, :, 0:D],
                        func=mybir.ActivationFunctionType.Copy,
                    )
                    mem_sbs.append(mem_sb)

                    # item 1-of-4 select, in place in g2; scalar engine
                    # copies the result into ne[..., 2D:3D] (frees g2 early)
                    ne_sb = nepool.tile([P, g * 3 * D], bf, tag="ne")
                    ne3 = ne_sb[:].rearrange("p (g c) -> p g c", g=g)
                    g2q = g2[:, q * g : (q + 1) * g, :]
                    nc.vector.copy_predicated(
                        out=g2q[:, :, 0 : 2 * D],
                        mask=is_sbs[0][:, q * g : (q + 1) * g]
                             .unsqueeze(2).broadcast_to([P, g, 2 * D]),
                        data=g2q[:, :, 2 * D : 4 * D],
                    )
                    nc.vector.copy_predicated(
                        out=g2q[:, :, 0:D],
                        mask=is_sbs[1][:, q * g : (q + 1) * g]
                             .unsqueeze(2).broadcast_to([P, g, D]),
                        data=g2q[:, :, D : 2 * D],
                    )
                    nc.scalar.activation(
                        out=ne3[:, :, 2 * D : 3 * D],
                        in_=g2q[:, :, 0:D],
                        func=mybir.ActivationFunctionType.Copy,
                    )
                    ne_sbs.append(ne_sb)

                # --- math phase ------------------------------------------
                for q in range(SB):
                    gi = g0 + q
                    GL = bgls[q]
                    C = g * GL
                    mem_sb = mem_sbs[q]
                    ne_sb = ne_sbs[q]
                    ne3 = ne_sb[:].rearrange("p (g c) -> p g c", g=g)

                    # itemT via PE transpose (bf16 PSUM), then v = W^T @ item
                    itemT_ps = ppool.tile([D, GP], bf, tag="itemT",
                                          space="PSUM")
                    for j in range(g):
                        nc.tensor.transpose(
                            out=itemT_ps[:, j * P : (j + 1) * P],
                            in_=ne3[:, j, 2 * D : 3 * D],
                            identity=id_sb[:],
                        )
                    itemT_sb = wpool.tile([D, GP], bf, tag="itemT")
                    nc.scalar.activation(
                        out=itemT_sb[:],
                        in_=itemT_ps[:],
                        func=mybir.ActivationFunctionType.Copy,
                    )

                    v_ps = ppoolv.tile([P, g * D], dt.float32, tag="v",
                                       space="PSUM")
                    for j in range(g):
                        nc.tensor.matmul(
                            v_ps[:, j * D : (j + 1) * D],
                            lhsT=itemT_sb[:, j * P : (j + 1) * P],
                            rhs=wt_sb[:],
                            start=True,
                            stop=True,
                        )
                    v_sb = wpool.tile([P, g * D], bf, tag="vsb")
                    nc.scalar.activation(
                        out=v_sb[:],
                        in_=v_ps[:],
                        func=mybir.ActivationFunctionType.Copy,
                    )

                    # scores = sum_d mem * v  (X-reduce over d)
                    mem4 = mem_sb[:].rearrange("p (g m d) -> p g m d",
                                               g=g, m=GL)
                    v_b = (
                        v_sb[:]
                        .rearrange("p (g d) -> p g d", g=g)
                        .unsqueeze(2)
                        .broadcast_to([P, g, GL, D])
                    )
                    prod_sb = prpool.tile([P, GM * D], bf, tag="prod")
                    prod4 = prod_sb[:].rearrange("p (g m d) -> p g m d",
                                                 g=g, m=M)[:, :, :GL, :]
                    nc.vector.tensor_mul(out=prod4, in0=mem4, in1=v_b)

                    scores_sb = wpool.tile([P, GM], dt.float32, tag="scores")
                    sc3 = scores_sb[:].rearrange("p (g m) -> p g m", g=g)
                    nc.vector.reduce_sum(
                        out=sc3[:, :, :GL], in_=prod4,
                        axis=mybir.AxisListType.X,
                    )

                    # w = (scores + b_bil) * mask
                    w_sb = wpool.tile([P, GM], bf, tag="w")
                    w3 = w_sb[:].rearrange("p (g m) -> p g m", g=g)
                    m3 = bt["mask"][:].rearrange(
                        "p (q g m) -> p q g m", q=SB, g=g
                    )[:, q]
                    nc.vector.scalar_tensor_tensor(
                        out=w3[:, :, :GL],
                        in0=sc3[:, :, :GL],
                        scalar=bbil_sb[:, :1],
                        in1=m3[:, :, :GL],
                        op0=mybir.AluOpType.add,
                        op1=mybir.AluOpType.mult,
                    )

                    # prod2 = w * mem, then halving-tree sum over m -> fu
                    w_b = w3[:, :, :GL].unsqueeze(3).broadcast_to([P, g, GL, D])
                    nc.vector.tensor_mul(out=prod4, in0=mem4, in1=w_b)

                    steps = _tree_steps(GL)
                    prod4 = prod_sb[:].rearrange("p (g m d) -> p g m d",
                                                 g=g, m=M)[:, :, :GL, :]
                    for si, (dlen, off) in enumerate(steps):
                        last = si == len(steps) - 1
                        dst = (ne3[:, :, D : 2 * D].unsqueeze(2)
                               if last and dlen == 1
                               else prod4[:, :, :dlen, :])
                        nc.vector.tensor_add(
                            out=dst,
                            in0=prod4[:, :, :dlen, :],
                            in1=prod4[:, :, off : off + dlen, :],
                        )
                    if not steps:  # GL == 1
                        nc.vector.tensor_copy(
                            out=ne3[:, :, D : 2 * D],
                            in_=prod4[:, :, 0, :],
                        )

                    # ne[..., 0:D] = fu * item
                    nc.vector.tensor_mul(
                        out=ne3[:, :, 0:D],
                        in0=ne3[:, :, D : 2 * D],
                        in1=ne3[:, :, 2 * D : 3 * D],
                    )

                    # MLP head: neT -> h = relu(W1 neT + b1) -> y
                    neT_ps = ppool.tile([3 * D, GP], bf, tag="neT",
                                        space="PSUM")
                    for j in range(g):
                        nc.tensor.transpose(
                            out=neT_ps[:, j * P : (j + 1) * P],
                            in_=ne3[:, j, :],
                            identity=id_sb[:],
                        )
                    neT_sb = wpool.tile([3 * D, GP], bf, tag="neTs")
                    nc.scalar.activation(
                        out=neT_sb[:],
                        in_=neT_ps[:],
                        func=mybir.ActivationFunctionType.Copy,
                    )

                    hT_ps = ppool.tile([8, GP], dt.float32, tag="hT",
                                       space="PSUM")
                    nc.tensor.matmul(
                        hT_ps[:],
                        lhsT=w1_sb[:],
                        rhs=neT_sb[:],
                        start=True,
                        stop=True,
                    )
                    hT_sb = wpool.tile([8, GP], bf, tag="hTs")
                    nc.scalar.activation(
                        out=hT_sb[:],
                        in_=hT_ps[:],
                        func=mybir.ActivationFunctionType.Relu,
                        bias=b1_sb[:, :1],
                    )

                    yT_ps = ppool.tile([1, GP], dt.float32, tag="yT",
                                       space="PSUM")
                    nc.tensor.matmul(
                        yT_ps[:],
                        lhsT=w2_sb[:],
                        rhs=hT_sb[:],
                        start=True,
                        stop=True,
                    )
                    y_sb = iopool.tile([1, GP], dt.float32, tag="y")
                    nc.scalar.activation(
                        out=y_sb[:],
                        in_=yT_ps[:],
                        func=mybir.ActivationFunctionType.Sigmoid,
                        bias=b2_sb[:1, :1],
                    )
                    nc.sync.dma_start(
                        out=y_out[gi * g : (gi + 1) * g, :], in_=y_sb[:]
                    )

    nc.compile()
    return nc


def _lengths_from_mask(mask_b):
    mm = np.asarray(mask_b, dtype=bool)
    pos = np.arange(1, M + 1, dtype=np.int32)
    return (mm * pos[None, :]).max(axis=1).astype(np.int32)


def prepare(item_inputs, member_ids, member_mask, n_cores=N_CORES):
    L = _lengths_from_mask(member_mask)
    order = np.argsort(-L, kind="stable")
    n = len(L)
    bc = n // n_cores
    nt = bc // P
    Ls = L[order]
    prof = [int(max(1, Ls[t * P * n_cores])) for t in range(nt)]
    return order, prof


def _wrap16(idv):
    """[n] int16 idx list -> [128, n/16] wrapped + replicated layout."""
    n = len(idv)
    w16 = idv.reshape(n // 16, 16).T
    return np.tile(w16, (8, 1))


def _make_in_maps(item_inputs, member_ids, member_mask, user_table, item_table,
                  W_bil, b_bil, W1, b1, W2, b2, order, prof, g=G):
    import ml_dtypes

    bf = ml_dtypes.bfloat16
    item_inputs = np.asarray(item_inputs).astype(np.int32).reshape(-1)
    member_ids = np.asarray(member_ids).astype(np.int32)
    mask_f = np.asarray(member_mask).astype(bf)
    user4 = np.ascontiguousarray(
        np.asarray(user_table, dtype=np.float32).astype(bf)
        .reshape(NU // 4, 4 * D)
    )
    item4 = np.ascontiguousarray(
        np.asarray(item_table, dtype=np.float32).astype(bf)
        .reshape(NI // 4, 4 * D)
    )
    w_bil_t = np.ascontiguousarray(np.asarray(W_bil, dtype=np.float32).T
                                   .astype(bf))
    w1_t = np.ascontiguousarray(np.asarray(W1, dtype=np.float32).T.astype(bf))
    w2_t = np.ascontiguousarray(np.asarray(W2, dtype=np.float32).T.astype(bf))
    b1_c = np.asarray(b1, dtype=np.float32).reshape(8, 1)
    b2_c = np.asarray(b2, dtype=np.float32).reshape(1, 1)
    bbil_c = np.full((P, 1), np.asarray(b_bil, dtype=np.float32).reshape(-1)[0],
                     dtype=np.float32)
    ident = np.eye(P, dtype=np.float32).astype(bf)

    gls = _group_gl(prof, g)
    ngroups = len(gls)

    in_maps = []
    for c in range(N_CORES):
        rows = order[c::N_CORES]
        mi = member_ids[rows]              # [bc, M]
        ii = item_inputs[rows]             # [bc]
        idx_parts, m23, modd, ip = [], [], [], []
        is23, isodd = [], []
        for gi in range(ngroups):
            GL = gls[gi]
            blk = mi[gi * g * P : (gi + 1) * g * P, :GL]     # [g*P, GL]
            b4 = blk.reshape(g, P, GL)
            idv = np.transpose(b4, (0, 2, 1)).reshape(-1)     # (j,m,p) order
            idx_parts.append(_wrap16((idv >> 2).astype(np.int16)))
            sub = (np.transpose(b4, (0, 2, 1)) & 3)           # [g, GL, P]
            subm = np.transpose(sub, (2, 0, 1)).reshape(P, g * GL)  # [p,(j,m)]
            m23.append((subm >= 2).astype(np.uint8))
            modd.append((subm & 1).astype(np.uint8))
            ib = ii[gi * g * P : (gi + 1) * g * P].reshape(g, P)
            iv = ib.reshape(-1)                                # (j,p) order
            ip.append(_wrap16((iv >> 2).astype(np.int16)))
            isub = (ib & 3).T                                  # [P, g]
            is23.append((isub >= 2).astype(np.uint8))
            isodd.append((isub & 1).astype(np.uint8))
        in_maps.append({
            "ids16": np.concatenate(idx_parts, axis=1),
            "iid16": np.concatenate(ip, axis=1),
            "msel23": np.concatenate(m23, axis=1),
            "mselodd": np.concatenate(modd, axis=1),
            "isel23": np.concatenate(is23, axis=1),
            "iselodd": np.concatenate(isodd, axis=1),
            "mask": np.ascontiguousarray(mask_f[rows]),
            "user4": user4,
            "item4": item4,
            "w_bil_t": w_bil_t,
            "w1_t": w1_t,
            "w2_t": w2_t,
            "b1": b1_c,
            "b2": b2_c,
            "bbil": bbil_c,
            "ident": ident,
        })
    return in_maps


def _get_compiled(prof):
    key = tuple(prof)
    if key not in _COMPILED:
        _COMPILED[key] = build_kernel(BC, G, prof=list(prof))
    return _COMPILED[key]


def run_on_hw(nc, in_maps, trace=False):
    from concourse import bass_utils

    return bass_utils.run_bass_kernel_spmd(
        nc, in_maps, core_ids=list(range(N_CORES)), trace=trace
    )


def kernel(item_inputs, member_ids, member_mask, user_table, item_table,
           W_bil, b_bil, W1, b1, W2, b2):
    order, prof = prepare(item_inputs, member_ids, member_mask)
    nc = _get_compiled(prof)
    in_maps = _make_in_maps(item_inputs, member_ids, member_mask, user_table,
                            item_table, W_bil, b_bil, W1, b1, W2, b2, order, prof)
    res = run_on_hw(nc, in_maps, trace=False)
    y = np.empty(B, dtype=np.float32)
    for c in range(N_CORES):
        y[order[c::N_CORES]] = res.results[c]["y"].reshape(BC)
    return y.reshape(B, 1)
